# revision 4
# baseline (speedup 1.0000x reference)
"""Trainium2 Bass kernel for nn_ATL_Layer_19284403159353.

Data-parallel over (t, wq) across 8 NeuronCores: cores 0-3 take t=0,
cores 4-7 take t=1, each with a 19-wq slice (one overlapping wq on the
last core of each t; the host drops the duplicate row).

Per core:
  - 1x1 conv + BN + LeakyReLU(0.2) embedding. BN scale is folded into
    the conv weight on the host; the BN shift is applied on-chip via
    y' = (psum + shift) + 4*relu(psum + shift) = 5*leaky(psum + shift),
    whose scale cancels after column L2 normalization.
  - Column L2 normalization of embedded query/support and raw support.
  - Two Gram matmuls in fp32r (full PE rate, inputs pre-rounded on the
    host bit-exactly to the hardware fp32r format): f_x from embedded,
    match from raw (raw-query norm folded in as a per-partition scale).
  - AEA gate: per-position 2-layer MLP threshold cv, then
    sigmoid(50*(f_x - cv)) with the L1 denominator accumulated by the
    scalar engine's accum_out, gated sum over each way block via a
    fused DVE scalar_tensor_tensor with accum_out.
Output per core: [1900, 5] way-block sums; the host does the final mean
over hw_q / shot and assembles the [2, 75, 5] score tensor.
"""
import numpy as np
import concourse.bacc as bacc
import concourse.tile as tile
import concourse.mybir as mybir
from concourse.bass_utils import run_bass_kernel_spmd

F32 = mybir.dt.float32
F32R = mybir.dt.float32r
AF = mybir.ActivationFunctionType
OP = mybir.AluOpType
AX = mybir.AxisListType

T, WQ, WS, C, HWX = 2, 75, 25, 640, 100
WAY, SHOT, HID = 5, 5, 40
NCH = C // 128                    # 5 contraction chunks
KS = WS * HWX                     # 2500 support positions
WAYB = SHOT * HWX                 # 500 = one way block
WQL = 19                          # wq per core (1 overlap on cores 3, 7)
POS = WQL * HWX                   # 1900 query positions per core
OUTP = 1920                       # padded to 15 x 128
SCALE_VALUE = 30.0
ATT = 50.0
NORM_EPS = 1e-12
BN_EPS = 1e-5
SUPER = [(0, 384), (384, 384), (768, 384), (1152, 384), (1536, 364)]
RANGES = [(0, 19), (19, 38), (38, 57), (56, 75)]


def _round_f32r(x: np.ndarray) -> np.ndarray:
    """Host-side fp32 -> fp32r rounding, bit-exact with the on-chip cast
    (round-to-nearest-even to an 11-bit mantissa, low 12 bits cleared)."""
    u = np.ascontiguousarray(x, dtype=np.float32).view(np.uint32)
    r = (u + 0x7FF + ((u >> 12) & 1)) & np.uint32(0xFFFFF000)
    return r.view(np.float32)


def _build():
    nc = bacc.Bacc("TRN2", target_bir_lowering=False)

    q = nc.dram_tensor("q", [C, POS], F32R, kind="ExternalInput")
    s = nc.dram_tensor("s", [C, KS], F32R, kind="ExternalInput")
    wf = nc.dram_tensor("wf", [C, C], F32R, kind="ExternalInput")     # (W*inv).T
    w1 = nc.dram_tensor("w1", [C, HID], F32R, kind="ExternalInput")   # psi_w1
    shifts = nc.dram_tensor("shifts", [2, NCH, 128], F32, kind="ExternalInput")
    rows = nc.dram_tensor("rows", [1, 81], F32, kind="ExternalInput")  # b1|w2/5|b2
    out = nc.dram_tensor("out", [OUTP, WAY], F32, kind="ExternalOutput")

    with tile.TileContext(nc) as tc:
        with tc.tile_pool(name="wpool", bufs=1) as wp, \
             tc.tile_pool(name="spool", bufs=1) as sp, \
             tc.tile_pool(name="qpool", bufs=2) as qp, \
             tc.tile_pool(name="hot", bufs=2) as hp, \
             tc.tile_pool(name="cfxp", bufs=4) as cp, \
             tc.tile_pool(name="ps_emb", bufs=2, space="PSUM") as pse, \
             tc.tile_pool(name="ps_g", bufs=2, space="PSUM") as psg, \
             tc.tile_pool(name="ps_small", bufs=2, space="PSUM") as pss:

            # ---------------- weights / constants ----------------
            wf_sb = wp.tile([128, NCH * C], F32R, tag="wf_sb")
            nc.sync.dma_start(wf_sb[:], wf.rearrange("(c p) o -> p c o", p=128))
            w1_sb = wp.tile([128, NCH * HID], F32R, tag="w1_sb")
            nc.sync.dma_start(w1_sb[:], w1.rearrange("(c p) h -> p c h", p=128))
            shift_sb = wp.tile([128, 2 * NCH], F32, tag="shift_sb")
            nc.sync.dma_start(shift_sb[:], shifts.rearrange("a c p -> p a c"))
            rows_f = wp.tile([1, 81], F32, tag="rows_f")
            nc.sync.dma_start(rows_f[:], rows[:, :])
            rbc = wp.tile([128, 81], F32, tag="rbc")
            nc.gpsimd.partition_broadcast(rbc[:], rows_f[:])
            b1_bc = rbc[:, 0:HID]
            w2_bc = rbc[:, HID:2 * HID]
            b2_col = rbc[:, 80:81]

            ones_f = wp.tile([128, 1], F32, tag="ones_f")
            nc.vector.memset(ones_f[:], 1.0)
            ones_r1 = wp.tile([128, 1], F32R, tag="ones_r1")
            nc.vector.tensor_copy(ones_r1[:], ones_f[:])
            ones_f2 = wp.tile([128, 2], F32, tag="ones_f2")
            nc.vector.memset(ones_f2[:], 1.0)
            ones_r2 = wp.tile([128, 2], F32R, tag="ones_r2")
            nc.vector.tensor_copy(ones_r2[:], ones_f2[:])

            def wfch(ci, oj):
                return wf_sb[:, ci * C + oj * 128: ci * C + (oj + 1) * 128]

            def embed_drain(psum_ap, oj, r4_ap, dst_ap):
                # y' = (psum + shift) + 4*relu(psum + shift) = 5*leaky
                nc.scalar.activation(r4_ap, psum_ap, AF.Relu,
                                     bias=shift_sb[:, NCH + oj:NCH + oj + 1],
                                     scale=4.0)
                nc.vector.scalar_tensor_tensor(
                    out=dst_ap, in0=psum_ap,
                    scalar=shift_sb[:, oj:oj + 1],
                    in1=r4_ap, op0=OP.add, op1=OP.add)

            # ---------------- support startup ----------------
            s_sb = sp.tile([128, NCH * KS], F32R, tag="s_sb")
            nc.sync.dma_start(s_sb[:], s.rearrange("(c p) n -> p c n", p=128))
            ws_sb = sp.tile([128, NCH * KS], F32R, tag="ws_sb")

            def sch(ci, k0, w):
                return s_sb[:, ci * KS + k0: ci * KS + k0 + w]

            def wsch(ci, k0, w):
                return ws_sb[:, ci * KS + k0: ci * KS + k0 + w]

            with tc.tile_pool(name="stpool", bufs=2) as stp:
                for oj in range(NCH):
                    for kt in range(NCH):
                        pe_t = pse.tile([128, 512], F32, tag="emb",
                                        name=f"se{oj}_{kt}")
                        for ci in range(NCH):
                            nc.tensor.matmul(
                                pe_t[:, :WAYB], wfch(ci, oj),
                                sch(ci, kt * WAYB, WAYB),
                                start=(ci == 0), stop=(ci == NCH - 1))
                        r4_t = stp.tile([128, 512], F32, tag="r4s",
                                        name=f"r4s{oj}_{kt}")
                        embed_drain(pe_t[:, :WAYB], oj, r4_t[:, :WAYB],
                                    wsch(oj, kt * WAYB, WAYB))

                # row-form column norms + in-place normalize (ws_sb, s_sb)
                for mat, chf in (("ws", wsch), ("s", sch)):
                    for kt in range(NCH):
                        pn = pse.tile([128, 512], F32, tag="emb",
                                      name=f"n{mat}{kt}")
                        for ci in range(NCH):
                            sq_t = stp.tile([128, 512], F32R, tag="sq",
                                            name=f"sq{mat}{kt}_{ci}")
                            nc.scalar.square(sq_t[:, :WAYB],
                                             chf(ci, kt * WAYB, WAYB))
                            nc.tensor.matmul(pn[:1, :WAYB], ones_r1[:],
                                             sq_t[:, :WAYB],
                                             start=(ci == 0),
                                             stop=(ci == NCH - 1))
                        rown = stp.tile([1, 512], F32, tag="rown",
                                        name=f"ro{mat}{kt}")
                        nc.scalar.sqrt(rown[:, :WAYB], pn[:1, :WAYB])
                        nc.vector.tensor_scalar_max(rown[:, :WAYB],
                                                    rown[:, :WAYB], NORM_EPS)
                        rinv = stp.tile([1, 512], F32, tag="rinv",
                                        name=f"ri{mat}{kt}")
                        nc.vector.reciprocal(rinv[:, :WAYB], rown[:, :WAYB])
                        bct = stp.tile([128, 512], F32, tag="bct",
                                       name=f"bc{mat}{kt}")
                        nc.gpsimd.partition_broadcast(bct[:, :WAYB],
                                                      rinv[:, :WAYB])
                        for ci in range(NCH):
                            nc.vector.tensor_mul(chf(ci, kt * WAYB, WAYB),
                                                 chf(ci, kt * WAYB, WAYB),
                                                 bct[:, :WAYB])

            # ---------------- query stream + hot loop ----------------
            junk = hp.tile([128, WAYB], F32, tag="junk")
            junk40 = hp.tile([128, HID], F32, tag="junk40")

            pt0 = 0
            for st_i, (q0, w_st) in enumerate(SUPER):
                q_sb = qp.tile([128, NCH * 384], F32R, tag="q_sb",
                               name=f"q{st_i}")
                nc.sync.dma_start(
                    q_sb[:, :NCH * w_st].rearrange("p (c n) -> p c n", c=NCH),
                    q[:, q0:q0 + w_st].rearrange("(c p) n -> p c n", p=128))
                wq_sb = qp.tile([128, NCH * 384], F32R, tag="wq_sb",
                                name=f"wq{st_i}")

                def qch(ci, j0, w, _q=q_sb, _w=w_st):
                    return _q[:, ci * _w + j0: ci * _w + j0 + w]

                def wqch(ci, j0, w, _wq=wq_sb, _w=w_st):
                    return _wq[:, ci * _w + j0: ci * _w + j0 + w]

                for oj in range(NCH):
                    pe_t = pse.tile([128, 512], F32, tag="emb",
                                    name=f"qe{st_i}_{oj}")
                    for ci in range(NCH):
                        nc.tensor.matmul(pe_t[:, :w_st], wfch(ci, oj),
                                         qch(ci, 0, w_st),
                                         start=(ci == 0), stop=(ci == NCH - 1))
                    r4_t = qp.tile([128, 384], F32, tag="r4q",
                                   name=f"r4q{st_i}_{oj}")
                    embed_drain(pe_t[:, :w_st], oj, r4_t[:, :w_st],
                                wqch(oj, 0, w_st))

                # wq column norms (row form) + in-place normalize
                pn = pse.tile([128, 512], F32, tag="emb", name=f"qn{st_i}")
                for ci in range(NCH):
                    sq_t = qp.tile([128, 384], F32R, tag="sqw",
                                   name=f"sqw{st_i}_{ci}")
                    nc.scalar.square(sq_t[:, :w_st], wqch(ci, 0, w_st))
                    nc.tensor.matmul(pn[:1, :w_st], ones_r1[:], sq_t[:, :w_st],
                                     start=(ci == 0), stop=(ci == NCH - 1))
                rown = qp.tile([1, 384], F32, tag="qrow", name=f"qro{st_i}")
                nc.scalar.sqrt(rown[:, :w_st], pn[:1, :w_st])
                nc.vector.tensor_scalar_max(rown[:, :w_st], rown[:, :w_st],
                                            NORM_EPS)
                rinv = qp.tile([1, 384], F32, tag="qrinv", name=f"qri{st_i}")
                nc.vector.reciprocal(rinv[:, :w_st], rown[:, :w_st])
                bcq = qp.tile([128, 384], F32, tag="bcq", name=f"bcq{st_i}")
                nc.gpsimd.partition_broadcast(bcq[:, :w_st], rinv[:, :w_st])
                for ci in range(NCH):
                    nc.vector.tensor_mul(wqch(ci, 0, w_st), wqch(ci, 0, w_st),
                                         bcq[:, :w_st])

                for j0 in range(0, w_st, 128):
                    P = min(128, w_st - j0)
                    tn = f"t{pt0}"

                    # raw-q column norms (column form, f32r N=2)
                    sqp = hp.tile([128, NCH * 128], F32R, tag="sqp",
                                  name=f"sqp{tn}")
                    pc = pss.tile([128, 2], F32, tag="small", name=f"qcn{tn}")
                    for ci in range(NCH):
                        nc.scalar.square(sqp[:, ci * 128: ci * 128 + P],
                                         qch(ci, j0, P))
                        nc.tensor.matmul(pc[:P, :],
                                         sqp[:, ci * 128: ci * 128 + P],
                                         ones_r2[:],
                                         start=(ci == 0), stop=(ci == NCH - 1))
                    rqs = hp.tile([128, 1], F32, tag="rqs", name=f"rqs{tn}")
                    nc.scalar.sqrt(rqs[:P], pc[:P, 0:1])
                    nc.vector.tensor_scalar_max(rqs[:P], rqs[:P], NORM_EPS)
                    rq = hp.tile([128, 1], F32, tag="rq", name=f"rq{tn}")
                    nc.vector.reciprocal(rq[:P], rqs[:P])

                    # psi MLP -> sigmoid bias  (-15*sig(hid@w2+b2) - 25)
                    ph = pss.tile([128, HID], F32, tag="small", name=f"psi{tn}")
                    for ci in range(NCH):
                        nc.tensor.matmul(ph[:P, :], wqch(ci, j0, P),
                                         w1_sb[:, ci * HID:(ci + 1) * HID],
                                         start=(ci == 0), stop=(ci == NCH - 1))
                    t40 = hp.tile([128, HID], F32, tag="t40", name=f"t40{tn}")
                    nc.vector.tensor_add(t40[:P], ph[:P, :], b1_bc[:P])
                    r440 = hp.tile([128, HID], F32, tag="r440", name=f"r440{tn}")
                    nc.scalar.activation(r440[:P], t40[:P], AF.Relu,
                                         bias=0.0, scale=4.0)
                    hid5 = hp.tile([128, HID], F32, tag="hid5", name=f"hid5{tn}")
                    nc.vector.tensor_add(hid5[:P], t40[:P], r440[:P])
                    out2 = hp.tile([128, 1], F32, tag="out2", name=f"out2{tn}")
                    nc.vector.scalar_tensor_tensor(
                        out=junk40[:P], in0=hid5[:P], scalar=1.0,
                        in1=w2_bc[:P], op0=OP.mult, op1=OP.mult,
                        accum_out=out2[:P])
                    sigc = hp.tile([128, 1], F32, tag="sigc", name=f"sigc{tn}")
                    nc.scalar.activation(sigc[:P], out2[:P], AF.Sigmoid,
                                         bias=b2_col[:P], scale=1.0)
                    biaspp = hp.tile([128, 1], F32, tag="biaspp",
                                     name=f"bp{tn}")
                    nc.scalar.activation(biaspp[:P], sigc[:P], AF.Copy,
                                         bias=-25.0, scale=-15.0)

                    # gate loop over way blocks
                    den = hp.tile([128, WAY], F32, tag="den", name=f"den{tn}")
                    S = hp.tile([128, WAY], F32, tag="S", name=f"S{tn}")
                    for w in range(WAY):
                        g1 = psg.tile([128, WAYB], F32, tag="g1",
                                      name=f"g1{tn}_{w}")
                        for ci in range(NCH):
                            nc.tensor.matmul(g1[:P, :], wqch(ci, j0, P),
                                             wsch(ci, w * WAYB, WAYB),
                                             start=(ci == 0),
                                             stop=(ci == NCH - 1))
                        cfx = cp.tile([128, WAYB], F32, tag="cfx",
                                      name=f"cfx{tn}_{w}")
                        nc.scalar.activation(cfx[:P], g1[:P, :], AF.Sigmoid,
                                             bias=biaspp[:P], scale=ATT,
                                             accum_out=den[:P, w:w + 1])
                        g2 = psg.tile([128, WAYB], F32, tag="g2",
                                      name=f"g2{tn}_{w}")
                        for ci in range(NCH):
                            nc.tensor.matmul(g2[:P, :], qch(ci, j0, P),
                                             sch(ci, w * WAYB, WAYB),
                                             start=(ci == 0),
                                             stop=(ci == NCH - 1))
                        nc.vector.scalar_tensor_tensor(
                            out=junk[:P], in0=g2[:P, :], scalar=rq[:P],
                            in1=cfx[:P], op0=OP.mult, op1=OP.mult,
                            accum_out=S[:P, w:w + 1])

                    dtot = hp.tile([128, 1], F32, tag="dtot", name=f"dt{tn}")
                    nc.vector.reduce_sum(dtot[:P], den[:P, :], axis=AX.X)
                    nc.vector.tensor_scalar_max(dtot[:P], dtot[:P], NORM_EPS)
                    rden = hp.tile([128, 1], F32, tag="rden", name=f"rd{tn}")
                    nc.vector.reciprocal(rden[:P], dtot[:P])
                    R = hp.tile([128, WAY], F32, tag="R", name=f"R{tn}")
                    nc.vector.tensor_scalar_mul(R[:P], S[:P, :], rden[:P])
                    nc.sync.dma_start(out[q0 + j0: q0 + j0 + P, :], R[:P])
                    pt0 += 1
    nc.compile()
    return nc


def kernel(query_feat, support_feat, W_conv, bn_gamma, bn_beta, bn_mean,
           bn_var, psi_w1, psi_b1, psi_w2, psi_b2, way_num, shot_num):
    way = int(np.asarray(way_num))
    shot = int(np.asarray(shot_num))
    assert way == WAY and shot == SHOT, (way, shot)
    query_feat = np.asarray(query_feat, dtype=np.float32)
    support_feat = np.asarray(support_feat, dtype=np.float32)

    inv = np.asarray(bn_gamma, np.float32) / np.sqrt(
        np.asarray(bn_var, np.float32) + BN_EPS)
    shift = np.asarray(bn_beta, np.float32) - np.asarray(bn_mean, np.float32) * inv
    wf_host = _round_f32r((np.asarray(W_conv, np.float32) * inv[:, None]).T)
    w1_host = _round_f32r(np.asarray(psi_w1, np.float32))
    shifts_host = np.stack([shift.reshape(NCH, 128),
                            4.0 * shift.reshape(NCH, 128)], axis=0)
    rows_host = np.zeros((1, 81), np.float32)
    rows_host[0, :HID] = np.asarray(psi_b1, np.float32)
    rows_host[0, HID:2 * HID] = np.asarray(psi_w2, np.float32)[:, 0] / 5.0
    rows_host[0, 80] = np.asarray(psi_b2, np.float32).reshape(-1)[0]

    in_maps = []
    for core in range(8):
        t = core // 4
        lo, hi = RANGES[core % 4]
        q_host = _round_f32r(
            query_feat[t, lo:hi].reshape(WQL, C, HWX)
            .transpose(1, 0, 2).reshape(C, POS))
        s_host = _round_f32r(
            support_feat[t].reshape(WS, C, HWX)
            .transpose(1, 0, 2).reshape(C, KS))
        in_maps.append({
            "q": q_host, "s": s_host, "wf": wf_host, "w1": w1_host,
            "shifts": shifts_host, "rows": rows_host,
        })

    nc = _build()
    res = run_bass_kernel_spmd(nc, in_maps, core_ids=list(range(8)))
    global _last_results, _last_in_maps
    _last_results = res
    _last_in_maps = in_maps

    score = np.zeros((T, WQ, WAY), np.float32)
    coef = SCALE_VALUE / (HWX * SHOT)
    for core in range(8):
        t = core // 4
        lo, hi = RANGES[core % 4]
        R = res.results[core]["out"][:POS].reshape(WQL, HWX, WAY)
        sc = R.sum(axis=1) * coef
        if core % 4 == 3:
            score[t, lo + 1:hi] = sc[1:]
        else:
            score[t, lo:hi] = sc
    return score
